# revision 70
# baseline (speedup 1.0000x reference)
"""Trainium2 Bass kernel for nn_Generator_34127810134219 (gnn_message_passing).

Strategy
--------
The reference relmod builds a [B,N,N] score matrix S = c*x@x^T (diag masked)
and computes wr*(S@U)/N + x.  Algebraically (verified to 4e-7 rel err):

    S@U = c*( x @ (x^T U) - ||x_i||^2 * U_i )

which collapses O(B*N^2*D) work into O(B*N*D^2).

Hybrid sharding (v2, 78.6us vs the 94.4us batch-parallel v1):
- Phase 1 (fc1 -> bn1 -> fc2 -> bn2 -> fc3+sigmoid) is sharded by NODES:
  each core owns 256 of the 2048 graph nodes for ALL 16 batches (batch map
  b = 2g + h: group g, column half h).  BatchNorm stats (per node, over
  batches x channels) are then fully core-local: a pack matmul PSUM-folds
  the two column halves, mean/var/rstd/A=g*rstd run in a compact [1,256]
  domain, and tiny matmuls broadcast A and mean back.  This removes the two
  bn1/bn2 stat exchanges (~8.5us serial latency each) of v1.
- One AllToAll reshards to batch-parallel form.  With the b = 2g + h map the
  sender side is a single flat [128,512] SBUF->DRAM copy (chunk k == enc
  partitions [16k,16k+16)); the receive side is 4 rectangular slice DMAs.
  Phase-2 node->group map: node 256j+n -> group j%4, column 256*(j//4)+n;
  bn4's gamma/beta and the output are permuted to match host-side (layout
  only).  Sigmoid garbage in channel rows c>=12 ships through but is
  provably inert (phi/ones_c16 select c<12; all weights are zero there).
- Phase 2/3 (4x relmod, fc4 -> bn4 -> fc5 -> fc67) run batch-parallel;
  bn4 stats use the [8,512] AllGather stat exchange (both pack sums land in
  one PSUM tile via column-padded fold matrices -> single store DMA).

Cost-model scheduling notes (TimelineSim):
- Every HWDGE DMA holds its issuing engine's SEQ until the shared HWDGE
  device frees, so all latency-critical loads ride the SP (sync) queue and
  compute-adjacent queues stay clean.
- PSUM banks serialize concurrent readers: bn-feeding fc layers emit as two
  half-width matmuls into different banks so the bias-add (DVE) and square
  (Act) halves run in parallel.
- A dep-pinned dummy Sigmoid right after bn2 absorbs the 1.3us Act table
  switch off the critical path; a second dummy rsqrt op switches back
  during the AllToAll window.
- Dependency-chained scratch matmuls keep the PE pstate ramped through the
  bn4 exchange windows (cold-start matmuls cost 3.7x).

On-chip layout: feature-major, group-packed [128,512] tiles, partition
16g+c.  All fc layers are block-diagonal 128x512 matmuls (f32r full-rate
PE).  The per-batch Gram matrix G = x^T U is built with PE transposes +
matmuls; per-group partials fold per batch as mask.(Phi^T P_m Phi).mask.

Host-side prep (layout only, no arithmetic): x is fed as [3,16,256] node
slices; bn1/bn2 gamma/beta as 256-node slices; bn4 vectors and the output
are index-permuted; the tiny fc/unary weights are packed into one [12,160]
tile, biases into [12,10], relmod scalars into [12].  The relmod scale
a = wr*ps*ph/N is folded into the unary weights ON DEVICE.

Timing model note: the single-core build (used for the cost-model timeline)
replaces each collective with 4 serialized DMAs ~ the 8-core mesh latency,
same convention as the measured baseline.
"""

import numpy as np

import concourse.bass as bass
import concourse.bacc as bacc
import concourse.tile as tile
import concourse.mybir as mybir
from concourse.bass_utils import run_bass_kernel_spmd
from concourse.masks import make_identity

FP32 = mybir.dt.float32
F32R = mybir.dt.float32r
AF = mybir.ActivationFunctionType
OP = mybir.AluOpType

B, N, F = 16, 2048, 3
D2, D4 = 6, 12
NCORES = 8
BPC = B // NCORES          # batches per core (phase 2)
NPC = N // NCORES          # nodes per core (phase 1)
T = BPC * N                # tokens per core
NG = 8                     # groups per core
L = T // NG                # free-dim length (512)
GS = 16                    # partition stride per group
EPS = 1e-5

# weight slot order inside the packed [12,160] tile (each slot is 16 cols)
W_ORDER = ["fc1_w", "fc2_w", "fc3_w", "u1_w", "u2_w", "u3_w", "u4_w",
           "fc4_w", "fc5_w", "fc67_w"]
# (out, in) dims per slot (fc67 packed as fc6 col 0, fc7 cols 1:3)
W_DIMS = {"fc1_w": (D2, F), "fc2_w": (D4, D2), "fc3_w": (D4, D4),
          "u1_w": (D4, D4), "u2_w": (D4, D4), "u3_w": (D4, D4),
          "u4_w": (D4, D4), "fc4_w": (D2, D4), "fc5_w": (F, D2),
          "fc67_w": (F, F)}
WSLOT = {name: i for i, name in enumerate(W_ORDER)}

# PE keep-warm chains: (wait_until_ms, count, label, psum_tag)
WARM_CFG = [(0.0550, 24, "bn4", "b3"),
            (0.0650, 16, "tail", "b3")]

# bn1/bn2 vectors arrive host-sliced to this core's 256 nodes; bn4 full
BN_LOCAL = ["bn1_g", "bn1_b", "bn2_g", "bn2_b"]
BN_FULL = ["bn4_g", "bn4_b"]


def _build(single_core=False):
    nc = bacc.Bacc(
        "TRN2",
        target_bir_lowering=False,
        debug=False,
        enable_asserts=False,
        num_devices=1 if single_core else NCORES,
    )

    x_d = nc.dram_tensor("x", [F, B, NPC], FP32, kind="ExternalInput")
    wall_d = nc.dram_tensor("wall", [D4, 16 * len(W_ORDER)], FP32,
                            kind="ExternalInput")
    ball_d = nc.dram_tensor("ball", [D4, len(W_ORDER)], FP32,
                            kind="ExternalInput")
    relsc_d = nc.dram_tensor("relsc", [12], FP32, kind="ExternalInput")
    prm = {name: nc.dram_tensor(name, [NPC], FP32, kind="ExternalInput")
           for name in BN_LOCAL}
    prm.update({name: nc.dram_tensor(name, [N], FP32, kind="ExternalInput")
                for name in BN_FULL})
    out_d = nc.dram_tensor("out", [BPC, F, N], FP32, kind="ExternalOutput")

    with tile.TileContext(nc) as tc:
        with (
            tc.tile_pool(name="consts", bufs=1) as cp,
            tc.tile_pool(name="sb", bufs=1) as sb,
            tc.tile_pool(name="pp", bufs=1, space="PSUM") as pp,
            tc.tile_pool(name="dram", bufs=1, space="DRAM") as dr,
        ):
            _emit(nc, tc, cp, sb, pp, dr, x_d, wall_d, ball_d, relsc_d,
                  prm, out_d, single_core=single_core)

    nc.compile()
    return nc


def _emit(nc, tc, cp, sb, pp, dr, x_d, wall_d, ball_d, relsc_d, prm, out_d,
          single_core=False):
    def mmr(out, lhsT, rhs, **kw):
        """float32r matmul: full-rate PE for fp32 bits (reduced mult precision)."""
        nc.tensor.matmul(out, lhsT.bitcast(F32R), rhs.bitcast(F32R), **kw)

    def r(ap):
        """f32r view for producer outputs feeding f32r matmuls (rounds)."""
        return ap.bitcast(F32R)

    eps_t = cp.tile([128, 1], FP32, name="eps_t")
    nc.gpsimd.memset(eps_t[:], EPS)
    # first ACT instruction resolves the table set once for the whole kernel
    actwarm = sb.tile([1, 1], FP32, name="actwarm")
    nc.scalar.activation(actwarm[:], eps_t[0:1, :], AF.Abs_reciprocal_sqrt)

    # ================= input / params (HWDGE, contiguous) ====================
    # phase-1 node-sharded tile: partition 16g+c <- channel c of batch g+8h,
    # column 256h+n (n = node within this core's 256-node slice)
    X = sb.tile([128, L], FP32, name="X")
    nc.vector.memset(X[:], 0.0)
    wall = cp.tile([D4, 16 * len(W_ORDER)], FP32, name="wall")
    nc.sync.dma_start(wall[:], wall_d[:, :])
    # ALL prologue loads go on the SP (sync) queue: a DMA holds its issuing
    # engine's SEQ until the shared HWDGE frees up, so scalar-queue DMAs here
    # would block Act compute (weight-prep copies) for microseconds.
    # phase-1 batch map: b = 2g + h -> per-channel DMA is 8 full 512-col rows
    for c in range(F):
        nc.sync.dma_start(
            X[:].rearrange("(g s) f -> g s f", s=GS)[:, c:c + 1, :].bitcast(F32R),
            x_d[c].rearrange("(g h) n -> g () (h n)", g=NG).bitcast(F32R))
    # hints keep these off the HWDGE slots that gate fc1 (wall + the 3 X DMAs)
    ball = cp.tile([D4, len(W_ORDER)], FP32, name="ball")
    with tc.tile_wait_until(0.0033):
        nc.sync.dma_start(ball[:], ball_d[:, :])
    # compact bn1/bn2 vectors: [1, 256] node slices, beta first (bnb_wide
    # broadcast needs it earlier than gamma)
    bnc = {}
    for i, name in enumerate(("bn1_b", "bn1_g", "bn2_b", "bn2_g")):
        t = cp.tile([1, NPC], FP32, name=f"c_{name}")
        with tc.tile_wait_until(0.004 + 0.0007 * i):
            nc.sync.dma_start(t[:].bitcast(F32R),
                              prm[name][:].rearrange("(u n) -> u n", u=1).bitcast(F32R))
        bnc[name] = t
    relsc = sb.tile([1, 12], FP32, name="relsc")
    with tc.tile_wait_until(0.0068):
        nc.sync.dma_start(relsc[:], relsc_d[:].rearrange("(u s) -> u s", u=1))
    # phase-2 tile, fully written by the AllToAll gather
    cur0 = sb.tile([128, L], FP32, name="cur0")

    # ================= affine-built base selectors (Pool engine) =============
    def affine_sel(t, pattern, cm):
        """t := 1.0 where cm*p + pattern.idx == 0 else 0."""
        nc.vector.memset(t, 0.0)
        nc.gpsimd.affine_select(
            out=t, in_=t, compare_op=OP.not_equal, fill=1.0,
            base=0, pattern=pattern, channel_multiplier=cm)

    # bc8[g, (g',c)] = [g'==g]
    bc8 = cp.tile([NG, 128], FP32, name="bc8")
    affine_sel(bc8[:].rearrange("p (g c) -> p g c", c=GS), [[1, NG], [0, GS]], -1)
    # RepSel12[ci', (g,ci)] = [ci==ci'] (ci'<12)
    rsel12 = cp.tile([D4, 128], FP32, name="rsel12")
    affine_sel(rsel12[:].rearrange("p (g c) -> p g c", c=GS), [[0, NG], [1, GS]], -1)
    # RepSel16
    rsel16 = cp.tile([GS, 128], FP32, name="rsel16")
    affine_sel(rsel16[:].rearrange("p (g c) -> p g c", c=GS), [[0, NG], [1, GS]], -1)

    # pack lhsT for local bn stats: 1/cnt on all partitions (garbage rows are 0)
    packc = {}
    for cnt in (96, 192):
        t0 = sb.tile([128, 1], FP32, name=f"packc0_{cnt}")
        nc.gpsimd.memset(t0[:], 1.0 / cnt)
        t = cp.tile([128, 1], FP32, name=f"packc_{cnt}")
        nc.vector.tensor_copy(r(t[:]), t0[:])
        packc[cnt] = t
    ones1 = cp.tile([1, 128], FP32, name="ones1")
    nc.gpsimd.memset(ones1[:], 1.0)
    ones1r = cp.tile([1, 128], FP32, name="ones1r")
    nc.vector.tensor_copy(r(ones1r[:]), ones1[:])

    # ================= PE-derived constant tiles =============================
    # mask_diag[(g,c),(g',c')] = [g==g']
    mask_ps = pp.tile([128, 128], FP32, name="mask_ps", tag="b0", padded_shape=[128, L])
    nc.tensor.matmul(mask_ps[:], bc8[:], bc8[:])
    mask_diag = cp.tile([128, 128], FP32, name="mask_diag")
    nc.scalar.activation(mask_diag[:], mask_ps[:], AF.Copy)

    # wide beta broadcast consts for bn1/bn2: b_bc[p, 256h+n] = beta[n]
    # (built lazily AFTER fc1's weight prep so the in-order PE queue never
    # head-blocks on the slow bnc vector DMAs ahead of fc1)
    bnb_wide = {}

    def build_bnb_wide(key):
        psb = pp.tile([128, L], FP32, name=f"psbb_{key}", tag="b5")
        mmr(psb[:, 0:NPC], ones1r[:], bnc[f"{key}_b"][:])
        mmr(psb[:, NPC:L], ones1r[:], bnc[f"{key}_b"][:])
        bb = cp.tile([128, L], FP32, name=f"bbw_{key}")
        nc.scalar.activation(bb[:], psb[:], AF.Copy)
        bnb_wide[key] = bb

    # ================= weights / biases ======================================
    WBD = {}
    BIAS = {}

    def finish_weight(wname, scale=None):
        s = WSLOT[wname]
        tp = pp.tile([GS, 128], FP32, name=f"wt_{wname}", tag="b6",
                     padded_shape=[128, L])
        nc.tensor.matmul(tp[:], wall[:, 16 * s:16 * (s + 1)], rsel12[:])
        ts = sb.tile([GS, 128], FP32, name=f"ws_{wname}", tag="wts")
        nc.scalar.activation(ts[:], tp[:], AF.Copy)
        sp = pp.tile([128, 128], FP32, name=f"wsp_{wname}", tag="b7",
                     padded_shape=[128, L])
        nc.tensor.matmul(sp[:], ts[:], rsel16[:])
        wt = cp.tile([128, 128], FP32, name=f"W_{wname}")
        if scale is None:
            nc.vector.tensor_tensor(r(wt[:]), sp[:], mask_diag[:], OP.mult)
        else:
            nc.vector.scalar_tensor_tensor(
                r(wt[:]), sp[:], scale[:], mask_diag[:], OP.mult, OP.mult)
        WBD[wname] = wt

    def finish_bias(wname, scale=None):
        s = WSLOT[wname]
        bps = pp.tile([128, 1], FP32, name=f"bps_{wname}", tag="b2",
                      padded_shape=[128, L])
        nc.tensor.matmul(bps[:], rsel12[:], ball[:, s:s + 1])
        bt = cp.tile([128, 1], FP32, name=f"bias_{wname}")
        nc.scalar.activation(bt[:], bps[:], AF.Copy)
        if scale is not None:
            bts = cp.tile([128, 1], FP32, name=f"biass_{wname}")
            nc.vector.tensor_tensor(bts[:], bt[:], scale[:], OP.mult)
            bt = bts
        BIAS[wname] = bt[:]

    # ================= helpers ===============================================
    def fc(w, src, name):
        ps = pp.tile([128, L], FP32, name=f"psfc_{name}", tag="b0")
        mmr(ps[:], w[:], src[:])
        return ps

    def fc_split(w, src, name):
        # halves in different PSUM banks so the bn bias-add/square can read
        # them concurrently from DVE and Act (PSUM banks serialize readers)
        ps_a = pp.tile([128, NPC], FP32, name=f"psfa_{name}", tag="b0",
                       padded_shape=[128, L])
        ps_b = pp.tile([128, NPC], FP32, name=f"psfb_{name}", tag="b5",
                       padded_shape=[128, L])
        mmr(ps_a[:], w[:], src[:, 0:NPC])
        mmr(ps_b[:], w[:], src[:, NPC:L])
        return ps_a, ps_b

    bn_mc = {}

    def bn_local(h_ps, bias, key, cnt, tag):
        h_ps_a, h_ps_b = h_ps
        """Node-sharded BN: stats are core-local (all 16 batches on core).

        Stats per node n: pack matmul sums partitions (batches x channels),
        PSUM-accumulating the two column halves; per-node mean/var/rstd run
        in a compact [1,256] domain, then A=g*rstd and mean are broadcast
        back via tiny matmuls.  hn = relu((hs - M_bc)*A_bc + b_bc).
        """
        # per-half bias-add/square on DVE (a) and Act (b) in parallel: the
        # halves live in different PSUM banks so the reads don't serialize
        hs = sb.tile([128, L], FP32, name=f"hs_{tag}")
        nc.vector.tensor_scalar_add(r(hs[:, 0:NPC]), h_ps_a[:], bias)
        nc.scalar.add(r(hs[:, NPC:L]), h_ps_b[:], bias)
        sq = sb.tile([128, L], FP32, name=f"sq_{tag}")
        nc.vector.tensor_tensor(r(sq[:, 0:NPC]), hs[:, 0:NPC], hs[:, 0:NPC],
                                OP.mult)
        nc.scalar.activation(r(sq[:, NPC:L]), hs[:, NPC:L], AF.Square)
        # mean (pack first: msq needs an extra Act hop vs ps_q read direct)
        ps_m = pp.tile([1, NPC], FP32, name=f"psm_{tag}", tag="b1",
                       padded_shape=[128, L])
        mmr(ps_m[:], packc[cnt][:], hs[:, 0:NPC], start=True, stop=False)
        mmr(ps_m[:], packc[cnt][:], hs[:, NPC:L], start=False, stop=True)
        ps_q = pp.tile([1, NPC], FP32, name=f"psq_{tag}", tag="b2",
                       padded_shape=[128, L])
        mmr(ps_q[:], packc[cnt][:], sq[:, 0:NPC], start=True, stop=False)
        mmr(ps_q[:], packc[cnt][:], sq[:, NPC:L], start=False, stop=True)
        msq = sb.tile([1, NPC], FP32, name=f"msq_{tag}")
        nc.scalar.activation(msq[:], ps_m[:], AF.Square)
        mc = sb.tile([1, NPC], FP32, name=f"mc_{tag}")
        nc.scalar.activation(r(mc[:]), ps_m[:], AF.Copy)
        var = sb.tile([1, NPC], FP32, name=f"var_{tag}")
        nc.vector.tensor_tensor(var[:], ps_q[:], msq[:], OP.subtract)
        rstd = sb.tile([1, NPC], FP32, name=f"rstd_{tag}")
        nc.scalar.activation(rstd[:], var[:], AF.Abs_reciprocal_sqrt,
                             bias=eps_t[0:1, :])
        bn_mc[key] = rstd
        A_c = sb.tile([1, NPC], FP32, name=f"Ac_{tag}")
        nc.vector.tensor_tensor(r(A_c[:]), rstd[:], bnc[f"{key}_g"][:], OP.mult)
        # broadcasts: M first (t1 runs while A broadcasts)
        psM = pp.tile([128, L], FP32, name=f"psM_{tag}", tag="b3")
        mmr(psM[:, 0:NPC], ones1r[:], mc[:])
        mmr(psM[:, NPC:L], ones1r[:], mc[:])
        psA = pp.tile([128, L], FP32, name=f"psA_{tag}", tag="b4")
        mmr(psA[:, 0:NPC], ones1r[:], A_c[:])
        mmr(psA[:, NPC:L], ones1r[:], A_c[:])
        t1 = sb.tile([128, L], FP32, name=f"t1_{tag}")
        nc.vector.tensor_tensor(t1[:], hs[:], psM[:], OP.subtract)
        t2 = sb.tile([128, L], FP32, name=f"t2_{tag}")
        nc.vector.tensor_tensor(t2[:], t1[:], psA[:], OP.mult)
        t3 = sb.tile([128, L], FP32, name=f"t3_{tag}")
        nc.vector.tensor_tensor(t3[:], t2[:], bnb_wide[key][:], OP.add)
        hn = sb.tile([128, L], FP32, name=f"hn_{tag}")
        nc.vector.tensor_relu(r(hn[:]), t3[:])
        return hn

    def bn_send(h_ps, bias, tag):
        """fc PSUM -> biased hs + partial stats -> AllGather kickoff (bn4)."""
        h_ps_a, h_ps_b = h_ps
        hs = sb.tile([128, L], FP32, name=f"hs_{tag}")
        nc.vector.tensor_scalar_add(r(hs[:, 0:NPC]), h_ps_a[:], bias)
        nc.scalar.add(r(hs[:, NPC:L]), h_ps_b[:], bias)
        sq = sb.tile([128, L], FP32, name=f"sq_{tag}")
        nc.vector.tensor_tensor(r(sq[:, 0:NPC]), hs[:, 0:NPC], hs[:, 0:NPC],
                                OP.mult)
        nc.scalar.activation(r(sq[:, NPC:L]), hs[:, NPC:L], AF.Square)
        pk8 = pp.tile([8, L], FP32, name=f"pk8_{tag}", tag="b1",
                      padded_shape=[128, L])
        mmr(pk8[:, 0:NPC], of8a[:], hs[:, 0:NPC], start=True, stop=False)
        mmr(pk8[:, 0:NPC], of8b[:], sq[:, 0:NPC], start=False, stop=True)
        mmr(pk8[:, NPC:L], of8a[:], hs[:, NPC:L], start=True, stop=False)
        mmr(pk8[:, NPC:L], of8b[:], sq[:, NPC:L], start=False, stop=True)
        # single [8,L] staging tile -> ONE store DMA
        sk = sb.tile([8, L], FP32, name=f"sk_{tag}")
        nc.vector.tensor_copy(sk[:], pk8[:])
        cc_in = dr.tile([8, L], FP32, name=f"ccin_{tag}")
        cc_out = dr.tile([64, L], FP32, name=f"ccout_{tag}")
        nc.sync.dma_start(cc_in[:, :], sk[:])
        if single_core:
            # timing-only stand-in for the AllGather (TimelineSim path);
            # 4 serialized DMAs model the ~5us 8-core AllGather latency
            for rr in range(4):
                nc.sync.dma_start(cc_out[8 * rr:8 * rr + 8, :], cc_in[:])
        else:
            nc.gpsimd.collective_compute(
                "AllGather",
                OP.bypass,
                replica_groups=[list(range(NCORES))],
                ins=[cc_in.opt()],
                outs=[cc_out.opt()],
            )
        return hs, cc_out

    def bn_recv(state, key, cnt, tag):
        """Gathered stats -> bn(h) = a*(h-mean)+beta -> relu (bn4)."""
        hs, cc_out = state
        gath = sb.tile([64, L], FP32, name=f"gath_{tag}")
        nc.sync.dma_start(gath[:].bitcast(F32R), cc_out[:].bitcast(F32R))
        M_bc = pp.tile([128, L], FP32, name=f"Mbc_{tag}", tag="b4")
        mmr(M_bc[:], CM[cnt][:], gath[:])
        Q_bc = pp.tile([128, L], FP32, name=f"Qbc_{tag}", tag="b1")
        mmr(Q_bc[:], CQ[cnt][:], gath[:])
        msq = sb.tile([128, L], FP32, name=f"msq_{tag}")
        nc.scalar.activation(msq[:], M_bc[:], AF.Square)
        var = sb.tile([128, L], FP32, name=f"var_{tag}")
        nc.vector.tensor_tensor(var[:], Q_bc[:], msq[:], OP.subtract)
        t1 = sb.tile([128, L], FP32, name=f"t1_{tag}")
        nc.vector.tensor_tensor(t1[:], hs[:], M_bc[:], OP.subtract)
        rstd = sb.tile([128, L], FP32, name=f"rstd_{tag}")
        nc.scalar.activation(rstd[:], var[:], AF.Abs_reciprocal_sqrt,
                             bias=eps_t[:])
        a = sb.tile([128, L], FP32, name=f"a_{tag}")
        nc.vector.tensor_tensor(a[:], rstd[:], bng_bc[key][:], OP.mult)
        t2 = sb.tile([128, L], FP32, name=f"t2_{tag}")
        nc.vector.tensor_tensor(t2[:], t1[:], a[:], OP.mult)
        t3 = sb.tile([128, L], FP32, name=f"t3_{tag}")
        nc.vector.tensor_tensor(t3[:], t2[:], bnb_bc[key][:], OP.add)
        hn = sb.tile([128, L], FP32, name=f"hn_{tag}")
        nc.vector.tensor_relu(r(hn[:]), t3[:])
        return hn

    def relmod(cur, wu, bu, idx):
        # U' = a*relu(unary(cur)) via the pre-scaled wu/bu
        psU = pp.tile([128, L], FP32, name=f"psU_{idx}", tag="b0")
        mmr(psU[:], wu[:], cur[:])
        # open the xG accumulator early with the +cur identity term so the
        # final output needs only ONE more matmul (Gf) and ONE vector op
        psXG = pp.tile([128, L], FP32, name=f"psXG_{idx}", tag="b7")
        nc.tensor.matmul(psXG[:], identr[:].bitcast(F32R), cur[:].bitcast(F32R), start=True, stop=False)
        U = sb.tile([128, L], FP32, name=f"U_{idx}", tag="U")
        nc.scalar.activation(r(U[:]), psU[:], AF.Relu, bias=bu)
        # transposes of cur and U (4x 128-chunks each, f32r for 1.5cyc/row)
        pTc = pp.tile([128, 4 * 128], FP32, name=f"pTc_{idx}", tag="b1")
        pTu = pp.tile([128, 4 * 128], FP32, name=f"pTu_{idx}", tag="b2")
        for j in range(4):
            nc.tensor.transpose(
                pTc[:, 128 * j:128 * (j + 1)].bitcast(F32R),
                cur[:, 128 * j:128 * (j + 1)].bitcast(F32R),
                identr[:].bitcast(F32R))
        for j in range(4):
            nc.tensor.transpose(
                pTu[:, 128 * j:128 * (j + 1)].bitcast(F32R),
                U[:, 128 * j:128 * (j + 1)].bitcast(F32R),
                identr[:].bitcast(F32R))
        curT = sb.tile([128, 4 * 128], FP32, name=f"curT_{idx}", tag="curT")
        nc.scalar.activation(r(curT[:]), pTc[:], AF.Copy)
        # UT copied in halves so psG's accumulation starts one hop earlier
        UTa = sb.tile([128, 256], FP32, name=f"UTa_{idx}", tag="UTa")
        UTb = sb.tile([128, 256], FP32, name=f"UTb_{idx}", tag="UTb")
        nc.vector.tensor_copy(r(UTa[:]), pTu[:, 0:256])
        nc.vector.tensor_copy(r(UTb[:]), pTu[:, 256:512])
        # P' = sum_t U x cur  (per-group partials on diag blocks)
        psG = pp.tile([128, 128], FP32, name=f"psG_{idx}", tag="b4",
                      padded_shape=[128, L])
        for j in range(4):
            ut = UTa if j < 2 else UTb
            mmr(psG[:], ut[:, 128 * (j % 2):128 * (j % 2 + 1)],
                curT[:, 128 * j:128 * (j + 1)],
                start=(j == 0), stop=(j == 3))
        # s = sum_c cur^2 per token, broadcast to [128,L].  The whole s-branch
        # has ~1.3us slack to the final nxt, so it runs on Pool/Act to keep
        # DVE free for the critical UT copies and G-fold ladder.
        sq = sb.tile([128, L], FP32, name=f"rsq_{idx}", tag="rsq")
        nc.vector.tensor_tensor(r(sq[:]), cur[:], cur[:], OP.mult)
        psS = pp.tile([NG, L], FP32, name=f"psS_{idx}", tag="b5", padded_shape=[128, L])
        mmr(psS[:], ones_c16[:], sq[:])
        sS = sb.tile([NG, L], FP32, name=f"sS_{idx}", tag="sS")
        nc.vector.tensor_copy(r(sS[:]), psS[:])
        Sbc = pp.tile([128, L], FP32, name=f"Sbc_{idx}", tag="b3")
        mmr(Sbc[:], bc8r[:], sS[:])
        Pm = sb.tile([128, 128], FP32, name=f"Pm_{idx}", tag="Pm")
        nc.vector.tensor_tensor(r(Pm[:]), psG[:], mask_diag[:], OP.mult)
        # G_spread = Phi^T (P_m Phi);  P_m = Pm^T
        psM = pp.tile([128, 128], FP32, name=f"psM_{idx}", tag="b5",
                      padded_shape=[128, L])
        mmr(psM[:], Pm[:], phi[:])
        Ms = sb.tile([128, 128], FP32, name=f"Ms_{idx}", tag="Ms")
        nc.vector.tensor_copy(r(Ms[:]), psM[:])
        psG2 = pp.tile([128, 128], FP32, name=f"psG2_{idx}", tag="b6",
                       padded_shape=[128, L])
        mmr(psG2[:], phi[:], Ms[:])
        Gf = sb.tile([128, 128], FP32, name=f"Gf_{idx}", tag="Gf")
        nc.vector.tensor_tensor(r(Gf[:]), psG2[:], mask_diag[:], OP.mult)
        # xG + cur lands in the open accumulator
        mmr(psXG[:], Gf[:], cur[:], start=False, stop=True)
        # out = (xG + cur) - s*U   (a already folded into U).  sbc_s/w1 run
        # on Act/Pool so they never steal DVE slots from the UT copies.
        sbc_s = sb.tile([128, L], FP32, name=f"sbcs_{idx}", tag="sbcs")
        nc.scalar.activation(sbc_s[:], Sbc[:], AF.Copy)
        w1 = sb.tile([128, L], FP32, name=f"w1_{idx}", tag="w1")
        nc.gpsimd.tensor_tensor(w1[:], sbc_s[:], U[:], OP.mult)
        nxt = sb.tile([128, L], FP32, name=f"nxt_{idx}", tag="nxt", bufs=2)
        nc.vector.tensor_tensor(r(nxt[:]), psXG[:], w1[:], OP.subtract)
        return nxt

    # ================= phase 1: node-sharded =================================
    finish_weight("fc1_w")
    finish_bias("fc1_w")
    ps1 = fc_split(WBD["fc1_w"], X, "1")
    # beta broadcasts hinted to their use time (t3) so their Act copies never
    # block the bn stat chain's Square/rstd ops
    with tc.tile_wait_until(0.0085):
        build_bnb_wide("bn1")
    h1n = bn_local(ps1, BIAS["fc1_w"], "bn1", 96, "bn1")
    finish_weight("fc2_w")
    finish_bias("fc2_w")
    ps2 = fc_split(WBD["fc2_w"], h1n, "2")
    with tc.tile_wait_until(0.0145):
        build_bnb_wide("bn2")
    h2n = bn_local(ps2, BIAS["fc2_w"], "bn2", 192, "bn2")
    with tc.tile_wait_until(0.008):
        finish_weight("fc3_w")
        finish_bias("fc3_w")
    # dummy sigmoid pinned after bn2's last rsqrt-set Act op (reads its mc):
    # the auto-inserted Sigmoid table switch (1.3us) attaches HERE and runs
    # during bn2's DVE tail instead of serializing behind fc3 on the path
    sigwarm = sb.tile([1, 1], FP32, name="sigwarm")
    nc.scalar.activation(sigwarm[:], bn_mc["bn2"][:, 0:1], AF.Sigmoid)
    ps3 = fc(WBD["fc3_w"], h2n, "3")
    # full-tile sigmoid; the c>=12 garbage rows (sigmoid(0)=0.5) ship through
    # the AllToAll but are provably inert downstream: phi/ones_c16 are built
    # from c<12 selectors, so relmod keeps garbage confined to dead rows, and
    # every fc weight is zero on those rows.
    enc = sb.tile([128, L], FP32, name="enc")
    nc.scalar.activation(r(enc[:]), ps3[:], AF.Sigmoid, bias=BIAS["fc3_w"])
    # switch the Act tables back to the rsqrt set right after the sigmoid
    # (runs in the AllToAll window, so bn4's rstd doesn't pay it on-path)
    actwarm2 = sb.tile([1, 1], FP32, name="actwarm2")
    nc.scalar.activation(actwarm2[:], enc[0:1, 0:1],
                         AF.Abs_reciprocal_sqrt, bias=eps_t[0:1, :])

    # ================= AllToAll reshard: node -> batch sharding ==============
    # With b = 2g + h, sender chunk k (cc rows 16k..16k+16) is exactly enc
    # partitions [16k, 16k+16): the store is ONE flat [128,512] copy.
    cc_a2a_in = dr.tile([128, L], FP32, name="cc_a2a_in")
    cc_a2a_out = dr.tile([128, L], FP32, name="cc_a2a_out")
    nc.sync.dma_start(cc_a2a_in[:, :], enc[:])
    if single_core:
        # timing-only stand-in (same convention as the bn AllGather model)
        for rr in range(4):
            nc.sync.dma_start(cc_a2a_out[32 * rr:32 * rr + 32, :],
                              cc_a2a_in[32 * rr:32 * rr + 32, :])
    else:
        nc.gpsimd.collective_compute(
            "AllToAll",
            OP.bypass,
            replica_groups=[list(range(NCORES))],
            ins=[cc_a2a_in.opt()],
            outs=[cc_a2a_out.opt()],
        )

    # ================= deferred consts / phase-2 params (A2A window) =========
    # hinted past bn2's stat chain; the sigwarm trick already moved the
    # Sigmoid table switch off-path, so this only needs to stay clear of
    # bn1/bn2's Act ops while finishing the serial weight chain before relmod1
    dfr = tc.tile_wait_until(0.016)
    dfr.__enter__()
    # bc4[j, (g,c)] = [g%4==j]
    bc4 = cp.tile([4, 128], FP32, name="bc4")
    affine_sel(bc4[:].rearrange("p (h j c) -> p h j c", j=4, c=GS),
               [[0, 2], [1, 4], [0, GS]], -1)
    # bcB[b, (g,c)] = [g//4==b]
    bcB = cp.tile([2, 128], FP32, name="bcB")
    affine_sel(bcB[:].rearrange("p (b j c) -> p b j c", j=4, c=GS),
               [[1, 2], [0, 4], [0, GS]], -1)
    # s8m[j, (r,j')] = [j'==j]; s8q[j, (r,j')] = [j'==j+4]  (stat-row selectors)
    s8m = cp.tile([4, 64], FP32, name="s8m")
    affine_sel(s8m[:].rearrange("p (r j) -> p r j", j=8), [[0, 8], [1, 8]], -1)
    s8q = cp.tile([4, 64], FP32, name="s8q")
    nc.vector.memset(s8q[:], 0.0)
    nc.gpsimd.affine_select(
        out=s8q[:].rearrange("p (r j) -> p r j", j=8),
        in_=s8q[:].rearrange("p (r j) -> p r j", j=8),
        compare_op=OP.not_equal, fill=1.0,
        base=-4, pattern=[[0, 8], [1, 8]], channel_multiplier=-1)
    selb = cp.tile([128, 6], FP32, name="selb")
    affine_sel(selb[:].rearrange("p (b c) -> p b c", c=3), [[0, 2], [1, 3]], -1)

    ident128 = cp.tile([128, 128], FP32, name="ident128")
    make_identity(nc, ident128[:])
    identr = cp.tile([128, 128], FP32, name="identr")
    nc.vector.tensor_copy(identr[:].bitcast(F32R), ident128[:])

    # bn4 vectors in [4, L] quarter layout (SWDGE; after the collective on
    # the Pool queue - values needed only at ~60us)
    bnvec = {}
    for name in BN_FULL:
        t = cp.tile([4, L], FP32, name=f"v_{name}")
        nc.gpsimd.dma_start(t[:].bitcast(F32R),
                            prm[name][:].rearrange("(j t) -> j t", t=L).bitcast(F32R))
        bnvec[name] = t

    # onesfold [128,4] = bc4^T (bn4 send pack); of8a/of8b are its [128,8]
    # column-padded variants so the two send pack matmuls can ACCUMULATE into
    # one [8,L] PSUM tile (sums rows 0:4, sumsq rows 4:8) at partition 0
    of_ps = pp.tile([128, 4], FP32, name="of_ps", tag="b3", padded_shape=[128, L])
    nc.tensor.transpose(of_ps[:], bc4[:], ident128[0:4, 0:4])
    onesfold = cp.tile([128, 4], FP32, name="onesfold")
    nc.scalar.activation(r(onesfold[:]), of_ps[:], AF.Copy)
    of8a = cp.tile([128, 8], FP32, name="of8a")
    nc.vector.memset(of8a[:], 0.0)
    nc.vector.tensor_copy(r(of8a[:, 0:4]), onesfold[:])
    of8b = cp.tile([128, 8], FP32, name="of8b")
    nc.vector.memset(of8b[:], 0.0)
    nc.vector.tensor_copy(r(of8b[:, 4:8]), onesfold[:])
    # f32r-rounded copies of selectors
    bc4r = cp.tile([4, 128], FP32, name="bc4r")
    nc.vector.tensor_copy(r(bc4r[:]), bc4[:])
    bc8r = cp.tile([NG, 128], FP32, name="bc8r")
    nc.vector.tensor_copy(r(bc8r[:]), bc8[:])

    # relmod scale a_i = wr_i*ps_i*ph_i/N, broadcast to [128,1]
    scm = sb.tile([1, 4], FP32, name="scm")
    nc.vector.tensor_tensor(scm[:], relsc[:, 0:4], relsc[:, 4:8], OP.mult)
    nc.vector.tensor_tensor(scm[:], scm[:], relsc[:, 8:12], OP.mult)
    nc.vector.tensor_scalar_mul(scm[:], scm[:], 1.0 / N)
    a_r = []
    for i in range(4):
        pb = pp.tile([128, 1], FP32, name=f"psc_{i}", tag="b3",
                     padded_shape=[128, L])
        nc.tensor.matmul(pb[:], ones1[:], scm[:, i:i + 1])
        at = cp.tile([128, 1], FP32, name=f"a_r{i}")
        nc.scalar.activation(at[:], pb[:], AF.Copy)
        a_r.append(at)

    # fold+broadcast matrices for bn4 stats and the group-fold phi.
    # crep/ones_c16 select c<12 only, which keeps the cur0 garbage rows inert.
    crep_ps = pp.tile([128, 128], FP32, name="crep_ps", tag="b1",
                      padded_shape=[128, L])
    nc.tensor.matmul(crep_ps[:], rsel12[:], rsel12[:])
    crep = sb.tile([128, 128], FP32, name="crep")
    nc.scalar.activation(crep[:], crep_ps[:], AF.Copy)
    bmask_ps = pp.tile([128, 128], FP32, name="bmask_ps", tag="b2",
                       padded_shape=[128, L])
    nc.tensor.matmul(bmask_ps[:], bcB[:], bcB[:])
    phi = cp.tile([128, 128], FP32, name="phi")
    nc.vector.tensor_tensor(r(phi[:]), bmask_ps[:], crep[:], OP.mult)
    ones12 = cp.tile([D4, 1], FP32, name="ones12")
    nc.gpsimd.memset(ones12[:], 1.0)
    cm_ps = pp.tile([128, 1], FP32, name="cm_ps", tag="b5",
                    padded_shape=[128, L])
    nc.tensor.matmul(cm_ps[:], rsel12[:], ones12[:])
    colmask12 = cp.tile([128, 1], FP32, name="colmask12")
    nc.scalar.activation(colmask12[:], cm_ps[:], AF.Copy)
    oc_ps = pp.tile([128, NG], FP32, name="oc_ps", tag="b4",
                    padded_shape=[128, L])
    nc.tensor.transpose(oc_ps[:], bc8[:], ident128[0:NG, 0:NG])
    ones_c16 = cp.tile([128, NG], FP32, name="ones_c16")
    nc.vector.tensor_scalar_mul(r(ones_c16[:]), oc_ps[:], colmask12[:])
    CM, CQ = {}, {}
    for cnt in (96,):
        cmp_ = pp.tile([64, 128], FP32, name=f"cmps_{cnt}", tag="b6",
                       padded_shape=[128, L])
        nc.tensor.matmul(cmp_[:], s8m[:], bc4[:])
        cm = cp.tile([64, 128], FP32, name=f"CM_{cnt}")
        nc.scalar.activation(r(cm[:]), cmp_[:], AF.Copy, scale=1.0 / cnt)
        CM[cnt] = cm
        cqp = pp.tile([64, 128], FP32, name=f"cqps_{cnt}", tag="b7",
                      padded_shape=[128, L])
        nc.tensor.matmul(cqp[:], s8q[:], bc4[:])
        cq = cp.tile([64, 128], FP32, name=f"CQ_{cnt}")
        nc.scalar.activation(r(cq[:]), cqp[:], AF.Copy, scale=1.0 / cnt)
        CQ[cnt] = cq

    for i in range(4):
        finish_weight(f"u{i + 1}_w", scale=a_r[i])
        finish_bias(f"u{i + 1}_w", scale=a_r[i])
    finish_weight("fc4_w")
    finish_bias("fc4_w")
    finish_weight("fc5_w")
    finish_bias("fc5_w")
    finish_weight("fc67_w")
    finish_bias("fc67_w")

    # bn4 scale/shift broadcast [128, L]
    bnb_bc, bng_bc = {}, {}
    for k in ("bn4",):
        bps = pp.tile([128, L], FP32, name=f"bnbps_{k}", tag="b3")
        mmr(bps[:], bc4r[:], bnvec[f"{k}_b"][:])
        bsb = cp.tile([128, L], FP32, name=f"bnbbc_{k}")
        nc.scalar.activation(bsb[:], bps[:], AF.Copy)
        bnb_bc[k] = bsb
        gps = pp.tile([128, L], FP32, name=f"bngps_{k}", tag="b4")
        mmr(gps[:], bc4r[:], bnvec[f"{k}_g"][:])
        gsb = cp.tile([128, L], FP32, name=f"bngbc_{k}")
        nc.scalar.activation(r(gsb[:]), gps[:], AF.Copy)
        bng_bc[k] = gsb

    # b67[(b,c'),0] = fc67 bias per output channel (b-independent)
    b67ps = pp.tile([6, 1], FP32, name="b67ps", tag="b2", padded_shape=[128, L])
    nc.tensor.matmul(b67ps[:], selb[:], BIAS["fc67_w"])
    b67 = cp.tile([6, 1], FP32, name="b67")
    nc.scalar.activation(b67[:], b67ps[:], AF.Copy)
    # fc67 per-quarter column-slice weights
    w67 = WBD["fc67_w"][:].rearrange("p (b rest) -> p b rest", b=2)
    w67q = []
    for q in range(4):
        t = cp.tile([128, 6], FP32, name=f"w67q_{q}")
        nc.vector.tensor_copy(r(t[:]), w67[:, :, 16 * q:16 * q + F])
        w67q.append(t)
    dfr.__exit__(None, None, None)

    # ================= A2A gather -> phase 2 =================================
    # cc_out rows 16j+c = channel c of my batches from core j, cols 256h+n
    # (h = local batch).  Phase-2 node->group map: node 256j+n lives in group
    # 4*bl + (j%4), column 256*(j//4) + n; the bn4 vectors and the output are
    # permuted to match host-side (layout only).
    for bl in range(2):
        # both on the sync queue: a scalar-queue DMA waiting on HWDGE would
        # hold the Act SEQ and delay relmod1's activations
        nc.sync.dma_start(
            cur0[64 * bl:64 * bl + 64, :].rearrange(
                "p (beta n) -> p beta n", beta=2),
            cc_a2a_out[:].rearrange("(beta p) n -> p beta n", beta=2)[
                :, :, NPC * bl:NPC * bl + NPC])

    cur = cur0
    for i in range(4):
        cur = relmod(cur, WBD[f"u{i + 1}_w"], BIAS[f"u{i + 1}_w"], i)

    st4 = bn_send(fc_split(WBD["fc4_w"], cur, "4"), BIAS["fc4_w"], "bn4")
    h4n = bn_recv(st4, "bn4", 96, "bn4")
    ps5 = fc(WBD["fc5_w"], h4n, "5")
    h5 = sb.tile([128, L], FP32, name="h5")
    nc.scalar.activation(r(h5[:]), ps5[:], AF.Relu, bias=BIAS["fc5_w"])
    # fused fc67: per quarter q, contract with the column slice of W_fc67
    # whose outputs are rows {64b+16q+c'} - the result lands directly in the
    # DRAM [3,2048]-per-batch layout, so the store is 2 contiguous DMAs
    oraw = sb.tile([6, 4 * L], FP32, name="oraw")
    psqs = [None] * 4
    for q in (3, 2, 1, 0):
        psq = pp.tile([6, L], FP32, name=f"psraw_{q}", tag=f"b{4 + q}",
                      padded_shape=[128, L])
        mmr(psq[:], w67q[q][:], h5[:])
        psqs[q] = psq
    for q in range(4):
        if q % 2 == 0:
            nc.scalar.add(oraw[:, L * q:L * (q + 1)], psqs[q][:], b67[:])
        else:
            nc.vector.tensor_scalar_add(oraw[:, L * q:L * (q + 1)], psqs[q][:], b67[:])
    for b in range(BPC):
        eng = nc.sync if b % 2 == 0 else nc.gpsimd
        eng.dma_start(
            out_d[b][:, :],
            oraw[F * b:F * b + F, :].rearrange("c (q f) -> c (q f)", q=4))

    # ---- PE keep-warm chains (emitted last = lowest priority) --------------
    # The cost model's PE runs at 1/3 rate after an idle gap and only reaches
    # full rate after ~3us of continuous work.  Fill the three long PE-idle
    # windows (AllToAll flight, bn4 exchange, bn4 recv) with dependency-
    # chained scratch matmuls so the real matmuls that follow (relmod1, fc5,
    # fc67) issue at full rate.  WAW on the shared b3 tag forms the chain.
    def pe_warm(ms, n, label, tag):
        with tc.tile_wait_until(ms):
            for i in range(n):
                pw = pp.tile([128, L], FP32, name=f"pw_{label}_{i}", tag=tag)
                mmr(pw[:], identr[:], X[:])

    for ms, n, label, tag in WARM_CFG:
        pe_warm(ms, n, label, tag)


_PROGRAM = None


def _get_program():
    global _PROGRAM
    if _PROGRAM is None:
        _PROGRAM = _build()
    return _PROGRAM


def _pack_params(inputs):
    """Host-side LAYOUT-ONLY packing of the tiny weights (no arithmetic)."""
    wall = np.zeros((D4, 16 * len(W_ORDER)), np.float32)
    ball = np.zeros((D4, len(W_ORDER)), np.float32)
    for s, name in enumerate(W_ORDER):
        slot = np.zeros((D4, 16), np.float32)
        if name == "fc67_w":
            slot[0:F, 0:1] = inputs["fc6_w"].T
            slot[0:F, 1:3] = inputs["fc7_w"].T
            ball[0:1, s] = inputs["fc6_b"]
            ball[1:3, s] = inputs["fc7_b"]
        else:
            w = inputs[name]
            o, i = w.shape
            slot[0:i, 0:o] = w.T
            ball[0:o, s] = inputs[name.replace("_w", "_b")]
        wall[:, 16 * s:16 * (s + 1)] = slot
    relsc = np.concatenate([
        np.concatenate([inputs[f"ps{i}"] for i in range(1, 5)]),
        np.concatenate([inputs[f"ph{i}"] for i in range(1, 5)]),
        np.concatenate([inputs[f"wr{i}"] for i in range(1, 5)]),
    ]).astype(np.float32)
    return (np.ascontiguousarray(wall), np.ascontiguousarray(ball),
            np.ascontiguousarray(relsc))


def run(inputs, trace=False, **kw):
    inputs = {k: np.asarray(v, np.float32) for k, v in inputs.items()}
    nc = _get_program()
    wall, ball, relsc = _pack_params(inputs)
    base = {"wall": wall, "ball": ball, "relsc": relsc}
    for name in BN_FULL:
        # phase-2 column map: tile col (gamma, beta, n) <-> node 256*(gamma+4*beta)+n
        v = inputs[name].reshape(2, 4, NPC).transpose(1, 0, 2).reshape(-1)
        base[name] = np.ascontiguousarray(v)
    in_maps = []
    for i in range(NCORES):
        m = dict(base)
        # phase-1 node shard: this core's 256 nodes, fed [3, 16, 256] so the
        # load DMA is 1KB-run contiguous (layout only)
        m["x"] = np.ascontiguousarray(
            inputs["x"][:, NPC * i:NPC * (i + 1), :].transpose(2, 0, 1))
        for name in BN_LOCAL:
            m[name] = np.ascontiguousarray(inputs[name][NPC * i:NPC * (i + 1)])
        in_maps.append(m)
    last_exc = None
    for attempt in range(3):
        try:
            res = run_bass_kernel_spmd(
                nc, in_maps, core_ids=list(range(NCORES)), trace=trace, **kw)
            break
        except Exception as e:  # transient NRT_EXEC_UNIT_UNRECOVERABLE flakes
            last_exc = e
            import time
            time.sleep(5)
    else:
        raise last_exc
    def unpermute(o):
        # tile col (gamma, beta, n) -> node 256*(gamma + 4*beta) + n
        return o.reshape(BPC, F, 4, 2, NPC).transpose(0, 1, 3, 2, 4).reshape(
            BPC, F, N)
    out = np.concatenate(
        [unpermute(res.results[i]["out"]).transpose(0, 2, 1)
         for i in range(NCORES)],
        axis=0)
    return np.ascontiguousarray(out), res


def kernel(**inputs) -> np.ndarray:
    out, _ = run(inputs)
    return out
